# revision 4
# baseline (speedup 1.0000x reference)
"""CMC triplet-score + memory-bank momentum update on 8 Trainium2 cores.

Problem shape: B=128 queries x 2 modalities, K+1=4097 gathered rows per
query from two 1M x 128 f32 memory banks, L2 triplet scores, plus a
128-row momentum scatter-update of each bank.

Sharding: data-parallel over the batch. Core c owns batch rows
[16c, 16c+16). Each core receives compacted copies of the bank rows its
queries touch (np.unique per 4-query group, indices remapped to int16),
gathers 4224 rows per (query, bank) on-device via dma_gather
(InstDMAGatherAnt), and computes distances/scores with DVE/ACT. The 16
updated bank rows per core are computed on-device and merged into a
host-side copy of the banks.

Device layout notes:
 - dma_gather places gathered row j on partition j%128, free block
   j//128, so dis tiles come out as [128, 33] per query with
   k = c*128 + p. Indices are wrapped int16: idx j at [j%16, j//16],
   replicated to 128 partitions.
 - Tables are per-4-query groups ("quarters") so the remapped index
   space (<= 4*4097 = 16388) always fits int16.
 - l/ab features are replicated across all 128 partitions host-side
   ([128, 16*128]) so the per-query feature vector is available on every
   partition for the subtract.
 - score output is written in tile layout [128, 16*33]; host transposes
   back to [16, 4097].
"""

import numpy as np

import concourse.bass as bass
import concourse.mybir as mybir
import concourse.tile as tile
from concourse import bacc
from concourse.bass_utils import run_bass_kernel_spmd

P = 128
D = 128
NCORES = 8
B = 128
K1 = 4097  # K+1
BPC = B // NCORES  # 16 batch rows per core
NQ = 4  # table groups ("quarters") per core
BPQ = BPC // NQ  # 4 batch rows per quarter
KT = (K1 + P - 1) // P  # 33 k-tiles of 128 rows
KPAD = KT * P  # 4224
KCOLS = BPC * KT  # 528
SWRAP = KPAD // 16  # 264 wrapped-index cols per query
TQ = BPQ * K1  # 16388 table rows per quarter (max uniques)
EPS = 1e-07
MARGIN = 1.0


def build_nc(
    bpc=BPC, kt=KT, nq=NQ, tq=TQ, square_on_act=False, enable_asserts=False
):
    bpq = bpc // nq
    kcols = bpc * kt
    kpad = kt * P
    swrap = kpad // 16
    frep_w = bpc * D
    f32, i32 = mybir.dt.float32, mybir.dt.int32
    i16 = mybir.dt.int16
    AX = mybir.AxisListType
    OP = mybir.AluOpType
    AF = mybir.ActivationFunctionType

    nc = bacc.Bacc(
        "TRN2",
        target_bir_lowering=False,
        debug=False,
        enable_asserts=enable_asserts,
    )
    tbls = {
        (name, q): nc.dram_tensor(
            f"table_{name}_{q}", [tq, D], f32, kind="ExternalInput"
        ).ap()
        for name in ("l", "ab")
        for q in range(nq)
    }
    idx16 = nc.dram_tensor("idx16", [P, bpc * swrap], i16, kind="ExternalInput").ap()
    lrep = nc.dram_tensor("lrep", [P, frep_w], f32, kind="ExternalInput").ap()
    abrep = nc.dram_tensor("abrep", [P, frep_w], f32, kind="ExternalInput").ap()
    l16 = nc.dram_tensor("l16", [bpc, D], f32, kind="ExternalInput").ap()
    ab16 = nc.dram_tensor("ab16", [bpc, D], f32, kind="ExternalInput").ap()
    yidx = nc.dram_tensor("yidx", [bpc, 1], i32, kind="ExternalInput").ap()
    score_l = nc.dram_tensor("score_l", [P, kcols], f32, kind="ExternalOutput").ap()
    score_ab = nc.dram_tensor("score_ab", [P, kcols], f32, kind="ExternalOutput").ap()
    upd_l = nc.dram_tensor("upd_l", [bpc, D], f32, kind="ExternalOutput").ap()
    upd_ab = nc.dram_tensor("upd_ab", [bpc, D], f32, kind="ExternalOutput").ap()

    with tile.TileContext(nc) as tc:
        with (
            tc.tile_pool(name="const", bufs=1) as cpool,
            tc.tile_pool(name="gab", bufs=2) as gab_pool,
            tc.tile_pool(name="gl", bufs=2) as gl_pool,
            tc.tile_pool(name="dis", bufs=4) as dis_pool,
            tc.tile_pool(name="pos", bufs=4) as pos_pool,
            tc.tile_pool(name="ps", bufs=2, space="PSUM") as ps_pool,
            tc.tile_pool(name="outp", bufs=1) as out_pool,
            tc.tile_pool(name="s16", bufs=1) as s16_pool,
        ):
            idx_sb = cpool.tile([P, bpc * swrap], i16, tag="idx")
            nc.sync.dma_start(out=idx_sb[:], in_=idx16)

            reps = {}
            for name, dram in (("l", lrep), ("ab", abrep)):
                sb = cpool.tile([P, frep_w], f32, tag=f"rep_{name}")
                nc.sync.dma_start(out=sb[:], in_=dram)
                reps[name] = sb

            ones_row = cpool.tile([1, P], f32, tag="ones")
            nc.vector.memset(ones_row[:], 1.0)

            # Normalize the replicated features in place: f /= (||f|| + eps).
            for name in ("l", "ab"):
                sb = reps[name]
                sq = cpool.tile([P, frep_w], f32, tag="normsq")
                nc.scalar.square(sq[:], sb[:])
                s2 = cpool.tile([P, bpc], f32, tag=f"s2_{name}")
                nc.vector.tensor_reduce(
                    out=s2[:],
                    in_=sq[:].rearrange("p (b d) -> p b d", d=D),
                    axis=AX.X,
                    op=OP.add,
                )
                nc.scalar.sqrt(s2[:], s2[:])
                nc.vector.tensor_scalar_add(s2[:], s2[:], EPS)
                rn = cpool.tile([P, bpc], f32, tag=f"rn_{name}")
                nc.vector.reciprocal(rn[:], s2[:])
                for b in range(bpc):
                    sl_ = sb[:, b * D : (b + 1) * D]
                    nc.vector.tensor_scalar(
                        out=sl_, in0=sl_, scalar1=rn[:, b : b + 1], scalar2=None,
                        op0=OP.mult,
                    )

            # Batch-row features [bpc, D] normalized for the bank update.
            f16n = {}
            for name, dram in (("l", l16), ("ab", ab16)):
                sb = s16_pool.tile([bpc, D], f32, tag=f"f16_{name}")
                nc.sync.dma_start(out=sb[:], in_=dram)
                scr = s16_pool.tile([bpc, D], f32, tag="scr16")
                n2 = s16_pool.tile([bpc, 1], f32, tag=f"n2_{name}")
                nc.scalar.activation(
                    out=scr[:], in_=sb[:], func=AF.Square, accum_out=n2[:]
                )
                nc.scalar.sqrt(n2[:], n2[:])
                nc.vector.tensor_scalar_add(n2[:], n2[:], EPS)
                rn = s16_pool.tile([bpc, 1], f32, tag=f"rn16_{name}")
                nc.vector.reciprocal(rn[:], n2[:])
                nc.vector.tensor_scalar(
                    out=sb[:], in0=sb[:], scalar1=rn[:, :1], scalar2=None, op0=OP.mult
                )
                f16n[name] = sb

            # Momentum update of the y rows: normalize(0.5*mem[y] + 0.5*f_n).
            y_sb = s16_pool.tile([bpc, 1], i32, tag="y")
            nc.sync.dma_start(out=y_sb[:], in_=yidx)
            for name, outdram in (("l", upd_l), ("ab", upd_ab)):
                ym = s16_pool.tile([bpc, D], f32, tag=f"ym_{name}")
                for q in range(nq):
                    nc.gpsimd.indirect_dma_start(
                        out=ym[q * bpq : (q + 1) * bpq, :],
                        out_offset=None,
                        in_=tbls[(name, q)],
                        in_offset=bass.IndirectOffsetOnAxis(
                            ap=y_sb[q * bpq : (q + 1) * bpq, :1], axis=0
                        ),
                    )
                t = s16_pool.tile([bpc, D], f32, tag=f"t_{name}")
                nc.vector.tensor_tensor(
                    out=t[:], in0=ym[:], in1=f16n[name][:], op=OP.add
                )
                usq = s16_pool.tile([bpc, D], f32, tag="scr16")
                un2 = s16_pool.tile([bpc, 1], f32, tag=f"un2_{name}")
                nc.scalar.activation(
                    out=usq[:], in_=t[:], func=AF.Square, scale=0.5, accum_out=un2[:]
                )
                nc.scalar.sqrt(un2[:], un2[:])
                urn = s16_pool.tile([bpc, 1], f32, tag=f"urn_{name}")
                nc.vector.reciprocal(urn[:], un2[:])
                upd_sb = s16_pool.tile([bpc, D], f32, tag=f"upd_{name}")
                nc.vector.tensor_scalar(
                    out=upd_sb[:], in0=t[:], scalar1=urn[:, :1], scalar2=0.5,
                    op0=OP.mult, op1=OP.mult,
                )
                nc.sync.dma_start(out=outdram, in_=upd_sb[:])

            # Main loop: per (query b, bank) gather 33x128 rows and compute
            # relu(1 + dis[0] - dis).
            sl_sb = out_pool.tile([P, kcols], f32, tag="sl")
            sab_sb = out_pool.tile([P, kcols], f32, tag="sab")
            for b in range(bpc):
                q = b // bpq
                for name, fsb, osb in (
                    ("ab", reps["l"], sl_sb),
                    ("l", reps["ab"], sab_sb),
                ):
                    pool = gab_pool if name == "ab" else gl_pool
                    w = pool.tile([P, kpad], f32)
                    w3 = w[:].rearrange("p (c d) -> p c d", d=D)
                    nc.gpsimd.dma_gather(
                        w3,
                        tbls[(name, q)],
                        idx_sb[:, b * swrap : (b + 1) * swrap],
                        kpad,
                        kpad,
                        D,
                        single_packet=False,
                    )
                    f3 = (
                        fsb[:, b * D : (b + 1) * D]
                        .rearrange("p (u d) -> p u d", u=1)
                        .to_broadcast([P, kt, D])
                    )
                    nc.vector.tensor_tensor(out=w3, in0=w3, in1=f3, op=OP.subtract)
                    if square_on_act:
                        nc.scalar.square(w[:], w[:])
                    else:
                        nc.vector.tensor_tensor(out=w[:], in0=w[:], in1=w[:], op=OP.mult)
                    dis = dis_pool.tile([P, kt], f32)
                    nc.vector.tensor_reduce(out=dis[:], in_=w3, axis=AX.X, op=OP.add)
                    nc.scalar.sqrt(dis[:], dis[:])
                    # Broadcast dis[k=0] (partition 0) to all partitions via PE.
                    pos_ps = ps_pool.tile([P, 1], f32)
                    nc.tensor.matmul(
                        out=pos_ps[:], lhsT=ones_row[:], rhs=dis[0:1, 0:1],
                        start=True, stop=True,
                    )
                    posp1 = pos_pool.tile([P, 1], f32)
                    nc.scalar.activation(
                        out=posp1[:], in_=pos_ps[:], func=AF.Copy, bias=1.0
                    )
                    nc.scalar.activation(
                        out=osb[:, b * kt : (b + 1) * kt], in_=dis[:], func=AF.Relu,
                        bias=posp1[:, :1], scale=-1.0,
                    )
            nc.sync.dma_start(out=score_l, in_=sl_sb[:])
            nc.sync.dma_start(out=score_ab, in_=sab_sb[:])
    nc.compile()
    return nc


_NC_CACHE = {}


def _get_nc():
    if "nc" not in _NC_CACHE:
        _NC_CACHE["nc"] = build_nc()
    return _NC_CACHE["nc"]


def wrap_idx(padded_row):
    """[KPAD] int array -> [128, SWRAP] int16 wrapped+replicated layout."""
    w = padded_row.reshape(SWRAP, 16).T.astype(np.int16)  # [16, SWRAP]
    return np.tile(w, (8, 1))  # [128, SWRAP]


def make_in_maps(l, ab, y, idx, memory_l, memory_ab):
    """Build the 8 per-core input dicts."""
    in_maps = []
    for c in range(NCORES):
        bs = slice(c * BPC, (c + 1) * BPC)
        chunk = idx[bs]  # [BPC, K1]
        ychunk = y[bs]
        m = {}
        idx16 = np.zeros((P, BPC * SWRAP), np.int16)
        ypos = np.zeros((BPC, 1), np.int32)
        for q in range(NQ):
            qrows = chunk[q * BPQ : (q + 1) * BPQ]  # [BPQ, K1]
            uniq = np.unique(qrows)
            remap = np.searchsorted(uniq, qrows)  # [BPQ, K1]
            ypos[q * BPQ : (q + 1) * BPQ, 0] = np.searchsorted(
                uniq, ychunk[q * BPQ : (q + 1) * BPQ]
            )
            for name, mem in (("l", memory_l), ("ab", memory_ab)):
                t = np.zeros((TQ, D), np.float32)
                t[: len(uniq)] = mem[uniq]
                m[f"table_{name}_{q}"] = t
            for j in range(BPQ):
                b = q * BPQ + j
                padded = np.zeros(KPAD, np.int64)
                padded[:K1] = remap[j]
                idx16[:, b * SWRAP : (b + 1) * SWRAP] = wrap_idx(padded)
        lchunk = l[bs].reshape(1, BPC * D)
        abchunk = ab[bs].reshape(1, BPC * D)
        m.update(
            {
                "idx16": idx16,
                "lrep": np.ascontiguousarray(np.broadcast_to(lchunk, (P, BPC * D))),
                "abrep": np.ascontiguousarray(np.broadcast_to(abchunk, (P, BPC * D))),
                "l16": np.ascontiguousarray(l[bs]),
                "ab16": np.ascontiguousarray(ab[bs]),
                "yidx": ypos,
            }
        )
        in_maps.append(m)
    return in_maps


def assemble(results, y, memory_l, memory_ab):
    out_l = np.empty((B, K1, 1), np.float32)
    out_ab = np.empty((B, K1, 1), np.float32)
    new_memory_l = memory_l.copy()
    new_memory_ab = memory_ab.copy()
    for c in range(NCORES):
        bs = slice(c * BPC, (c + 1) * BPC)
        for key, dst in (("score_l", out_l), ("score_ab", out_ab)):
            r = results[c][key]  # [P, KCOLS]
            t = r.reshape(P, BPC, KT).transpose(1, 2, 0).reshape(BPC, KPAD)[:, :K1]
            dst[bs] = t[:, :, None]
        new_memory_l[y[bs]] = results[c]["upd_l"]
        new_memory_ab[y[bs]] = results[c]["upd_ab"]
    return out_l, out_ab, new_memory_l, new_memory_ab


def kernel(l, ab, y, idx, memory_l, memory_ab, trace=False, _return_raw=False):
    l = np.ascontiguousarray(np.asarray(l), dtype=np.float32)
    ab = np.ascontiguousarray(np.asarray(ab), dtype=np.float32)
    y = np.ascontiguousarray(np.asarray(y), dtype=np.int32)
    idx = np.ascontiguousarray(np.asarray(idx), dtype=np.int32)
    memory_l = np.ascontiguousarray(np.asarray(memory_l), dtype=np.float32)
    memory_ab = np.ascontiguousarray(np.asarray(memory_ab), dtype=np.float32)

    nc = _get_nc()
    in_maps = make_in_maps(l, ab, y, idx, memory_l, memory_ab)
    br = run_bass_kernel_spmd(nc, in_maps, core_ids=list(range(NCORES)), trace=trace)
    out = assemble(br.results, y, memory_l, memory_ab)
    if _return_raw:
        return out, br
    return out


# revision 5
# speedup vs baseline: 1.0042x; 1.0042x over previous
"""CMC triplet-score + memory-bank momentum update on 8 Trainium2 cores.

Problem shape: B=128 queries x 2 modalities, K+1=4097 gathered rows per
query from two 1M x 128 f32 memory banks, L2 triplet scores, plus a
128-row momentum scatter-update of each bank.

Sharding: data-parallel over the batch. Core c owns batch rows
[16c, 16c+16). Each core receives compacted copies of the bank rows its
queries touch (np.unique per 4-query group, indices remapped to int16),
gathers 4224 rows per (query, bank) on-device via dma_gather
(InstDMAGatherAnt), and computes distances/scores with DVE/ACT. The 16
updated bank rows per core are computed on-device and merged into a
host-side copy of the banks.

Device layout notes:
 - dma_gather places gathered row j on partition j%128, free block
   j//128, so dis tiles come out as [128, 33] per query with
   k = c*128 + p. Indices are wrapped int16: idx j at [j%16, j//16],
   replicated to 128 partitions.
 - Tables are per-4-query groups ("quarters") so the remapped index
   space (<= 4*4097 = 16388) always fits int16.
 - l/ab features are replicated across all 128 partitions host-side
   ([128, 16*128]) so the per-query feature vector is available on every
   partition for the subtract.
 - score output is written in tile layout [128, 16*33]; host transposes
   back to [16, 4097].
"""

import numpy as np

import concourse.bass as bass
import concourse.mybir as mybir
import concourse.tile as tile
from concourse import bacc
from concourse.bass_utils import run_bass_kernel_spmd

P = 128
D = 128
NCORES = 8
B = 128
K1 = 4097  # K+1
BPC = B // NCORES  # 16 batch rows per core
NQ = 4  # table groups ("quarters") per core
BPQ = BPC // NQ  # 4 batch rows per quarter
KT = (K1 + P - 1) // P  # 33 k-tiles of 128 rows
KPAD = KT * P  # 4224
KCOLS = BPC * KT  # 528
SWRAP = KPAD // 16  # 264 wrapped-index cols per query
TQ = BPQ * K1  # 16388 table rows per quarter (max uniques)
EPS = 1e-07
MARGIN = 1.0


def build_nc(
    bpc=BPC, kt=KT, nq=NQ, tq=TQ, square_on_act=False, enable_asserts=False
):
    bpq = bpc // nq
    kcols = bpc * kt
    kpad = kt * P
    swrap = kpad // 16
    frep_w = bpc * D
    f32, i32 = mybir.dt.float32, mybir.dt.int32
    i16 = mybir.dt.int16
    AX = mybir.AxisListType
    OP = mybir.AluOpType
    AF = mybir.ActivationFunctionType

    nc = bacc.Bacc(
        "TRN2",
        target_bir_lowering=False,
        debug=False,
        enable_asserts=enable_asserts,
    )
    tbls = {
        (name, q): nc.dram_tensor(
            f"table_{name}_{q}", [tq, D], f32, kind="ExternalInput"
        ).ap()
        for name in ("l", "ab")
        for q in range(nq)
    }
    idx16 = nc.dram_tensor("idx16", [P, bpc * swrap], i16, kind="ExternalInput").ap()
    lrep = nc.dram_tensor("lrep", [P, frep_w], f32, kind="ExternalInput").ap()
    abrep = nc.dram_tensor("abrep", [P, frep_w], f32, kind="ExternalInput").ap()
    l16 = nc.dram_tensor("l16", [bpc, D], f32, kind="ExternalInput").ap()
    ab16 = nc.dram_tensor("ab16", [bpc, D], f32, kind="ExternalInput").ap()
    yidx = nc.dram_tensor("yidx", [bpc, 1], i32, kind="ExternalInput").ap()
    score_l = nc.dram_tensor("score_l", [P, kcols], f32, kind="ExternalOutput").ap()
    score_ab = nc.dram_tensor("score_ab", [P, kcols], f32, kind="ExternalOutput").ap()
    upd_l = nc.dram_tensor("upd_l", [bpc, D], f32, kind="ExternalOutput").ap()
    upd_ab = nc.dram_tensor("upd_ab", [bpc, D], f32, kind="ExternalOutput").ap()

    with tile.TileContext(nc) as tc:
        with (
            tc.tile_pool(name="const", bufs=1) as cpool,
            tc.tile_pool(name="gab", bufs=2) as gab_pool,
            tc.tile_pool(name="gl", bufs=2) as gl_pool,
            tc.tile_pool(name="dis", bufs=4) as dis_pool,
            tc.tile_pool(name="pos", bufs=4) as pos_pool,
            tc.tile_pool(name="ps", bufs=2, space="PSUM") as ps_pool,
            tc.tile_pool(name="outp", bufs=1) as out_pool,
            tc.tile_pool(name="s16", bufs=1) as s16_pool,
        ):
            idx_sb = cpool.tile([P, bpc * swrap], i16, tag="idx")
            nc.sync.dma_start(out=idx_sb[:], in_=idx16)

            reps = {}
            for name, dram in (("l", lrep), ("ab", abrep)):
                sb = cpool.tile([P, frep_w], f32, tag=f"rep_{name}")
                nc.sync.dma_start(out=sb[:], in_=dram)
                reps[name] = sb

            ones_row = cpool.tile([1, P], f32, tag="ones")
            nc.vector.memset(ones_row[:], 1.0)

            # Normalize the replicated features in place: f /= (||f|| + eps).
            for name in ("l", "ab"):
                sb = reps[name]
                sq = cpool.tile([P, frep_w], f32, tag="normsq")
                nc.scalar.square(sq[:], sb[:])
                s2 = cpool.tile([P, bpc], f32, tag=f"s2_{name}")
                nc.vector.tensor_reduce(
                    out=s2[:],
                    in_=sq[:].rearrange("p (b d) -> p b d", d=D),
                    axis=AX.X,
                    op=OP.add,
                )
                nc.scalar.sqrt(s2[:], s2[:])
                nc.vector.tensor_scalar_add(s2[:], s2[:], EPS)
                rn = cpool.tile([P, bpc], f32, tag=f"rn_{name}")
                nc.vector.reciprocal(rn[:], s2[:])
                for b in range(bpc):
                    sl_ = sb[:, b * D : (b + 1) * D]
                    nc.vector.tensor_scalar(
                        out=sl_, in0=sl_, scalar1=rn[:, b : b + 1], scalar2=None,
                        op0=OP.mult,
                    )

            # Momentum update of the y rows: normalize(0.5*mem[y] + 0.5*f_n).
            # Everything per (bank, quarter) with offset-0 tiles — indirect
            # DMA dests must not carry partition offsets on HW.
            for name, fdram, outdram in (
                ("l", l16, upd_l),
                ("ab", ab16, upd_ab),
            ):
                for q in range(nq):
                    qs = slice(q * bpq, (q + 1) * bpq)
                    fq = s16_pool.tile([bpq, D], f32, tag=f"f16_{name}_{q}")
                    nc.sync.dma_start(out=fq[:], in_=fdram[qs, :])
                    scr = s16_pool.tile([bpq, D], f32, tag=f"scr16_{q}")
                    n2 = s16_pool.tile([bpq, 1], f32, tag=f"n2_{name}_{q}")
                    nc.scalar.activation(
                        out=scr[:], in_=fq[:], func=AF.Square, accum_out=n2[:]
                    )
                    nc.scalar.sqrt(n2[:], n2[:])
                    nc.vector.tensor_scalar_add(n2[:], n2[:], EPS)
                    rn = s16_pool.tile([bpq, 1], f32, tag=f"rn16_{name}_{q}")
                    nc.vector.reciprocal(rn[:], n2[:])
                    nc.vector.tensor_scalar(
                        out=fq[:], in0=fq[:], scalar1=rn[:, :1], scalar2=None,
                        op0=OP.mult,
                    )
                    yq = s16_pool.tile([bpq, 1], i32, tag=f"y_{q}")
                    nc.sync.dma_start(out=yq[:], in_=yidx[qs, :])
                    ym = s16_pool.tile([bpq, D], f32, tag=f"ym_{name}_{q}")
                    nc.gpsimd.indirect_dma_start(
                        out=ym[:],
                        out_offset=None,
                        in_=tbls[(name, q)],
                        in_offset=bass.IndirectOffsetOnAxis(ap=yq[:, :1], axis=0),
                    )
                    t = s16_pool.tile([bpq, D], f32, tag=f"t_{name}_{q}")
                    nc.vector.tensor_tensor(
                        out=t[:], in0=ym[:], in1=fq[:], op=OP.add
                    )
                    usq = s16_pool.tile([bpq, D], f32, tag=f"scr16_{q}")
                    un2 = s16_pool.tile([bpq, 1], f32, tag=f"un2_{name}_{q}")
                    nc.scalar.activation(
                        out=usq[:], in_=t[:], func=AF.Square, scale=0.5,
                        accum_out=un2[:],
                    )
                    nc.scalar.sqrt(un2[:], un2[:])
                    urn = s16_pool.tile([bpq, 1], f32, tag=f"urn_{name}_{q}")
                    nc.vector.reciprocal(urn[:], un2[:])
                    upd_sb = s16_pool.tile([bpq, D], f32, tag=f"upd_{name}_{q}")
                    nc.vector.tensor_scalar(
                        out=upd_sb[:], in0=t[:], scalar1=urn[:, :1], scalar2=0.5,
                        op0=OP.mult, op1=OP.mult,
                    )
                    nc.sync.dma_start(out=outdram[qs, :], in_=upd_sb[:])

            # Main loop: per (query b, bank) gather 33x128 rows and compute
            # relu(1 + dis[0] - dis).
            sl_sb = out_pool.tile([P, kcols], f32, tag="sl")
            sab_sb = out_pool.tile([P, kcols], f32, tag="sab")
            for b in range(bpc):
                q = b // bpq
                for name, fsb, osb in (
                    ("ab", reps["l"], sl_sb),
                    ("l", reps["ab"], sab_sb),
                ):
                    pool = gab_pool if name == "ab" else gl_pool
                    w = pool.tile([P, kpad], f32)
                    w3 = w[:].rearrange("p (c d) -> p c d", d=D)
                    nc.gpsimd.dma_gather(
                        w3,
                        tbls[(name, q)],
                        idx_sb[:, b * swrap : (b + 1) * swrap],
                        kpad,
                        kpad,
                        D,
                        single_packet=False,
                    )
                    f3 = (
                        fsb[:, b * D : (b + 1) * D]
                        .rearrange("p (u d) -> p u d", u=1)
                        .to_broadcast([P, kt, D])
                    )
                    nc.vector.tensor_tensor(out=w3, in0=w3, in1=f3, op=OP.subtract)
                    if square_on_act:
                        nc.scalar.square(w[:], w[:])
                    else:
                        nc.vector.tensor_tensor(out=w[:], in0=w[:], in1=w[:], op=OP.mult)
                    dis = dis_pool.tile([P, kt], f32)
                    nc.vector.tensor_reduce(out=dis[:], in_=w3, axis=AX.X, op=OP.add)
                    nc.scalar.sqrt(dis[:], dis[:])
                    # Broadcast dis[k=0] (partition 0) to all partitions via PE.
                    pos_ps = ps_pool.tile([P, 1], f32)
                    nc.tensor.matmul(
                        out=pos_ps[:], lhsT=ones_row[:], rhs=dis[0:1, 0:1],
                        start=True, stop=True,
                    )
                    posp1 = pos_pool.tile([P, 1], f32)
                    nc.scalar.activation(
                        out=posp1[:], in_=pos_ps[:], func=AF.Copy, bias=1.0
                    )
                    nc.scalar.activation(
                        out=osb[:, b * kt : (b + 1) * kt], in_=dis[:], func=AF.Relu,
                        bias=posp1[:, :1], scale=-1.0,
                    )
            nc.sync.dma_start(out=score_l, in_=sl_sb[:])
            nc.sync.dma_start(out=score_ab, in_=sab_sb[:])
    nc.compile()
    return nc


_NC_CACHE = {}


def _get_nc():
    if "nc" not in _NC_CACHE:
        _NC_CACHE["nc"] = build_nc()
    return _NC_CACHE["nc"]


def wrap_idx(padded_row):
    """[KPAD] int array -> [128, SWRAP] int16 wrapped+replicated layout."""
    w = padded_row.reshape(SWRAP, 16).T.astype(np.int16)  # [16, SWRAP]
    return np.tile(w, (8, 1))  # [128, SWRAP]


def make_in_maps(l, ab, y, idx, memory_l, memory_ab):
    """Build the 8 per-core input dicts."""
    in_maps = []
    for c in range(NCORES):
        bs = slice(c * BPC, (c + 1) * BPC)
        chunk = idx[bs]  # [BPC, K1]
        ychunk = y[bs]
        m = {}
        idx16 = np.zeros((P, BPC * SWRAP), np.int16)
        ypos = np.zeros((BPC, 1), np.int32)
        for q in range(NQ):
            qrows = chunk[q * BPQ : (q + 1) * BPQ]  # [BPQ, K1]
            uniq = np.unique(qrows)
            remap = np.searchsorted(uniq, qrows)  # [BPQ, K1]
            ypos[q * BPQ : (q + 1) * BPQ, 0] = np.searchsorted(
                uniq, ychunk[q * BPQ : (q + 1) * BPQ]
            )
            for name, mem in (("l", memory_l), ("ab", memory_ab)):
                t = np.zeros((TQ, D), np.float32)
                t[: len(uniq)] = mem[uniq]
                m[f"table_{name}_{q}"] = t
            for j in range(BPQ):
                b = q * BPQ + j
                padded = np.zeros(KPAD, np.int64)
                padded[:K1] = remap[j]
                idx16[:, b * SWRAP : (b + 1) * SWRAP] = wrap_idx(padded)
        lchunk = l[bs].reshape(1, BPC * D)
        abchunk = ab[bs].reshape(1, BPC * D)
        m.update(
            {
                "idx16": idx16,
                "lrep": np.ascontiguousarray(np.broadcast_to(lchunk, (P, BPC * D))),
                "abrep": np.ascontiguousarray(np.broadcast_to(abchunk, (P, BPC * D))),
                "l16": np.ascontiguousarray(l[bs]),
                "ab16": np.ascontiguousarray(ab[bs]),
                "yidx": ypos,
            }
        )
        in_maps.append(m)
    return in_maps


def assemble(results, y, memory_l, memory_ab):
    out_l = np.empty((B, K1, 1), np.float32)
    out_ab = np.empty((B, K1, 1), np.float32)
    new_memory_l = memory_l.copy()
    new_memory_ab = memory_ab.copy()
    for c in range(NCORES):
        bs = slice(c * BPC, (c + 1) * BPC)
        for key, dst in (("score_l", out_l), ("score_ab", out_ab)):
            r = results[c][key]  # [P, KCOLS]
            t = r.reshape(P, BPC, KT).transpose(1, 2, 0).reshape(BPC, KPAD)[:, :K1]
            dst[bs] = t[:, :, None]
        new_memory_l[y[bs]] = results[c]["upd_l"]
        new_memory_ab[y[bs]] = results[c]["upd_ab"]
    return out_l, out_ab, new_memory_l, new_memory_ab


def kernel(l, ab, y, idx, memory_l, memory_ab, trace=False, _return_raw=False):
    l = np.ascontiguousarray(np.asarray(l), dtype=np.float32)
    ab = np.ascontiguousarray(np.asarray(ab), dtype=np.float32)
    y = np.ascontiguousarray(np.asarray(y), dtype=np.int32)
    idx = np.ascontiguousarray(np.asarray(idx), dtype=np.int32)
    memory_l = np.ascontiguousarray(np.asarray(memory_l), dtype=np.float32)
    memory_ab = np.ascontiguousarray(np.asarray(memory_ab), dtype=np.float32)

    nc = _get_nc()
    in_maps = make_in_maps(l, ab, y, idx, memory_l, memory_ab)
    br = run_bass_kernel_spmd(nc, in_maps, core_ids=list(range(NCORES)), trace=trace)
    out = assemble(br.results, y, memory_l, memory_ab)
    if _return_raw:
        return out, br
    return out


# revision 15
# speedup vs baseline: 1.6148x; 1.6081x over previous
"""CMC triplet-score + memory-bank momentum update on 8 Trainium2 cores.

Problem shape: B=128 queries x 2 modalities, K+1=4097 gathered rows per
query from two 1M x 128 f32 memory banks, L2 triplet scores, plus a
128-row momentum scatter-update of each bank.

Sharding: data-parallel over the batch. Core c owns batch rows
[16c, 16c+16). Each core receives compacted copies of the bank rows its
queries touch (np.unique per 4-query group, indices remapped to int16),
with the two banks' rows CONCATENATED into 1024-byte rows so one
dma_gather descriptor fetches both banks for a (query, k) pair. Gathers
run as prepare_only SWDGE preps + trigger_dma over 4 rotating SWDGE
queues so descriptor drain overlaps across queues and with compute.
Distances/scores run on DVE/ACT. The 16 updated bank rows per core are
computed on-device and merged into a host-side copy of the banks.

Device layout notes:
 - dma_gather places gathered row j on partition j%128, free block
   j//128. Row layout [mem_l | mem_ab], so the per-query gather tile is
   [128, 33, 256] and the reduced dis2 tile is [128, 66] with
   col 2c+0 = bank-l (scores out_ab), col 2c+1 = bank-ab (out_l),
   k = c*128 + p.
 - Indices are wrapped int16: idx j at [j%16, j//16], replicated to 128
   partitions. Tables are per-4-query groups so the remapped index
   space (<= 4*4097 = 16388) always fits int16.
 - Features are replicated across all 128 partitions host-side as
   [ab | l] pairs ([128, 16*256]) to match the gather-row layout.
 - score output is written in tile layout [128, 16*33]; host transposes
   back to [16, 4097].
"""

import numpy as np

import concourse.bass as bass
import concourse.mybir as mybir
import concourse.tile as tile
from concourse import bacc
from concourse.bass_utils import run_bass_kernel_spmd

P = 128
D = 128
D2 = 2 * D  # concatenated row width
NCORES = 8
B = 128
K1 = 4097  # K+1
BPC = B // NCORES  # 16 batch rows per core
NQ = 4  # table groups ("quarters") per core
BPQ = BPC // NQ  # 4 batch rows per quarter
KT = (K1 + P - 1) // P  # 33 k-tiles of 128 rows
KPAD = KT * P  # 4224
KCOLS = BPC * KT  # 528
SWRAP = KPAD // 16  # 264 wrapped-index cols per query
TQ = BPQ * K1  # 16388 table rows per quarter (max uniques)
NQUEUES = 4  # SWDGE queues for gather drain parallelism
EPS = 1e-07
MARGIN = 1.0


def build_nc(
    bpc=BPC, kt=KT, nq=NQ, tq=TQ, square_on_act=True, gather_bufs=2,
    enable_asserts=False,
):
    bpq = bpc // nq
    kcols = bpc * kt
    kpad = kt * P
    swrap = kpad // 16
    frep_w = bpc * D2
    f32, i32 = mybir.dt.float32, mybir.dt.int32
    i16 = mybir.dt.int16
    AX = mybir.AxisListType
    OP = mybir.AluOpType
    AF = mybir.ActivationFunctionType

    nc = bacc.Bacc(
        "TRN2",
        target_bir_lowering=False,
        debug=False,
        enable_asserts=enable_asserts,
        num_swdge_queues=NQUEUES,
    )
    tbls = {
        q: nc.dram_tensor(
            f"table_{q}", [tq, D2], f32, kind="ExternalInput"
        ).ap()
        for q in range(nq)
    }
    idx16 = nc.dram_tensor("idx16", [P, bpc * swrap], i16, kind="ExternalInput").ap()
    frep = nc.dram_tensor("frep", [P, frep_w], f32, kind="ExternalInput").ap()
    l16 = nc.dram_tensor("l16", [bpc, D], f32, kind="ExternalInput").ap()
    ab16 = nc.dram_tensor("ab16", [bpc, D], f32, kind="ExternalInput").ap()
    yidx = nc.dram_tensor("yidx", [bpc, 1], i32, kind="ExternalInput").ap()
    score_l = nc.dram_tensor("score_l", [P, kcols], f32, kind="ExternalOutput").ap()
    score_ab = nc.dram_tensor("score_ab", [P, kcols], f32, kind="ExternalOutput").ap()
    upd_l = nc.dram_tensor("upd_l", [bpc, D], f32, kind="ExternalOutput").ap()
    upd_ab = nc.dram_tensor("upd_ab", [bpc, D], f32, kind="ExternalOutput").ap()

    with tile.TileContext(nc) as tc:
        with (
            tc.tile_pool(name="const", bufs=1) as cpool,
            tc.tile_pool(name="gw", bufs=gather_bufs) as gpool,
            tc.tile_pool(name="dis", bufs=4) as dis_pool,
            tc.tile_pool(name="pos", bufs=4) as pos_pool,
            tc.tile_pool(name="ps", bufs=4, space="PSUM") as ps_pool,
            tc.tile_pool(name="outp", bufs=1) as out_pool,
            tc.tile_pool(name="s16", bufs=1) as s16_pool,
        ):
            idx_sb = cpool.tile([P, bpc * swrap], i16, tag="idx")
            nc.sync.dma_start(out=idx_sb[:], in_=idx16)

            fsb = cpool.tile([P, frep_w], f32, tag="frep")
            nc.sync.dma_start(out=fsb[:], in_=frep)

            ones_row = cpool.tile([1, P], f32, tag="ones")
            nc.vector.memset(ones_row[:], 1.0)

            # Normalize the 2*bpc replicated half-features in place.
            nhalf = 2 * bpc
            sq = cpool.tile([P, D], f32, tag="normsq")
            s2 = cpool.tile([P, nhalf], f32, tag="s2")
            for h in range(nhalf):
                nc.scalar.activation(
                    out=sq[:], in_=fsb[:, h * D : (h + 1) * D], func=AF.Square,
                    accum_out=s2[:, h : h + 1],
                )
            nc.scalar.sqrt(s2[:], s2[:])
            nc.vector.tensor_scalar_add(s2[:], s2[:], EPS)
            rn = cpool.tile([P, nhalf], f32, tag="rn")
            nc.vector.reciprocal(rn[:], s2[:])
            for h in range(nhalf):
                sl_ = fsb[:, h * D : (h + 1) * D]
                nc.vector.tensor_scalar(
                    out=sl_, in0=sl_, scalar1=rn[:, h : h + 1], scalar2=None,
                    op0=OP.mult,
                )

            # Momentum update of the y rows: normalize(0.5*mem[y] + 0.5*f_n).
            # Per-quarter tiles; indirect DMA dests must be offset-0 on HW.
            for q in range(nq):
                qs = slice(q * bpq, (q + 1) * bpq)
                yq = s16_pool.tile([bpq, 1], i32, tag=f"y_{q}")
                nc.sync.dma_start(out=yq[:], in_=yidx[qs, :])
                ym = s16_pool.tile([bpq, D2], f32, tag=f"ym_{q}")
                nc.gpsimd.indirect_dma_start(
                    out=ym[:],
                    out_offset=None,
                    in_=tbls[q],
                    in_offset=bass.IndirectOffsetOnAxis(ap=yq[:, :1], axis=0),
                )
                for name, fdram, outdram, half in (
                    ("l", l16, upd_l, 0),
                    ("ab", ab16, upd_ab, 1),
                ):
                    fq = s16_pool.tile([bpq, D], f32, tag=f"f16_{name}_{q}")
                    nc.sync.dma_start(out=fq[:], in_=fdram[qs, :])
                    scr = s16_pool.tile([bpq, D], f32, tag=f"scr16_{q}")
                    n2 = s16_pool.tile([bpq, 1], f32, tag=f"n2_{name}_{q}")
                    nc.scalar.activation(
                        out=scr[:], in_=fq[:], func=AF.Square, accum_out=n2[:]
                    )
                    nc.scalar.sqrt(n2[:], n2[:])
                    nc.vector.tensor_scalar_add(n2[:], n2[:], EPS)
                    rn16 = s16_pool.tile([bpq, 1], f32, tag=f"rn16_{name}_{q}")
                    nc.vector.reciprocal(rn16[:], n2[:])
                    nc.vector.tensor_scalar(
                        out=fq[:], in0=fq[:], scalar1=rn16[:, :1], scalar2=None,
                        op0=OP.mult,
                    )
                    t = s16_pool.tile([bpq, D], f32, tag=f"t_{name}_{q}")
                    nc.vector.tensor_tensor(
                        out=t[:], in0=ym[:, half * D : (half + 1) * D], in1=fq[:],
                        op=OP.add,
                    )
                    usq = s16_pool.tile([bpq, D], f32, tag=f"scr16_{q}")
                    un2 = s16_pool.tile([bpq, 1], f32, tag=f"un2_{name}_{q}")
                    nc.scalar.activation(
                        out=usq[:], in_=t[:], func=AF.Square, scale=0.5,
                        accum_out=un2[:],
                    )
                    nc.scalar.sqrt(un2[:], un2[:])
                    urn = s16_pool.tile([bpq, 1], f32, tag=f"urn_{name}_{q}")
                    nc.vector.reciprocal(urn[:], un2[:])
                    upd_sb = s16_pool.tile([bpq, D], f32, tag=f"upd_{name}_{q}")
                    nc.vector.tensor_scalar(
                        out=upd_sb[:], in0=t[:], scalar1=urn[:, :1], scalar2=0.5,
                        op0=OP.mult, op1=OP.mult,
                    )
                    nc.sync.dma_start(out=outdram[qs, :], in_=upd_sb[:])

            # Main loop: per query b gather 33x128 concatenated rows (both
            # banks) and compute relu(1 + dis[0] - dis) for each bank.
            sl_sb = out_pool.tile([P, kcols], f32, tag="sl")
            sab_sb = out_pool.tile([P, kcols], f32, tag="sab")
            for b in range(bpc):
                tq_idx = b // bpq
                queue = b % NQUEUES
                w = gpool.tile([P, kt * D2], f32)
                w3 = w[:].rearrange("p (c d) -> p c d", d=D2)
                nc.gpsimd.dma_gather(
                    w3,
                    tbls[tq_idx],
                    idx_sb[:, b * swrap : (b + 1) * swrap],
                    kpad,
                    kpad,
                    D2,
                    single_packet=False,
                    queue_num=queue,
                )
                f3 = (
                    fsb[:, b * D2 : (b + 1) * D2]
                    .rearrange("p (u d) -> p u d", u=1)
                    .to_broadcast([P, kt, D2])
                )
                nc.vector.tensor_tensor(out=w3, in0=w3, in1=f3, op=OP.subtract)
                if square_on_act:
                    nc.scalar.square(w[:], w[:])
                else:
                    nc.vector.tensor_tensor(out=w[:], in0=w[:], in1=w[:], op=OP.mult)
                dis = dis_pool.tile([P, 2 * kt], f32)
                nc.vector.tensor_reduce(
                    out=dis[:],
                    in_=w[:].rearrange("p (j d) -> p j d", d=D),
                    axis=AX.X,
                    op=OP.add,
                )
                nc.scalar.sqrt(dis[:], dis[:])
                # dis col 2c+0 = bank l (-> score_ab), 2c+1 = bank ab (-> score_l)
                for t_half, osb in ((0, sab_sb), (1, sl_sb)):
                    pos_ps = ps_pool.tile([P, 1], f32)
                    nc.tensor.matmul(
                        out=pos_ps[:], lhsT=ones_row[:],
                        rhs=dis[0:1, t_half : t_half + 1],
                        start=True, stop=True,
                    )
                    posp1 = pos_pool.tile([P, 1], f32)
                    nc.scalar.activation(
                        out=posp1[:], in_=pos_ps[:], func=AF.Copy, bias=1.0
                    )
                    din = (
                        dis[:]
                        .rearrange("p (c t) -> p c t", t=2)[:, :, t_half : t_half + 1]
                        .rearrange("p c u -> p (c u)")
                    )
                    nc.scalar.activation(
                        out=osb[:, b * kt : (b + 1) * kt], in_=din, func=AF.Relu,
                        bias=posp1[:, :1], scale=-1.0,
                    )
            nc.sync.dma_start(out=score_l, in_=sl_sb[:])
            nc.sync.dma_start(out=score_ab, in_=sab_sb[:])
    nc.compile()
    return nc


_NC_CACHE = {}


def _get_nc():
    if "nc" not in _NC_CACHE:
        _NC_CACHE["nc"] = build_nc()
    return _NC_CACHE["nc"]


def wrap_idx(padded_row):
    """[KPAD] int array -> [128, SWRAP] int16 wrapped+replicated layout."""
    w = padded_row.reshape(SWRAP, 16).T.astype(np.int16)  # [16, SWRAP]
    return np.tile(w, (8, 1))  # [128, SWRAP]


def make_in_maps(l, ab, y, idx, memory_l, memory_ab):
    """Build the 8 per-core input dicts."""
    in_maps = []
    for c in range(NCORES):
        bs = slice(c * BPC, (c + 1) * BPC)
        chunk = idx[bs]  # [BPC, K1]
        ychunk = y[bs]
        m = {}
        idx16 = np.zeros((P, BPC * SWRAP), np.int16)
        ypos = np.zeros((BPC, 1), np.int32)
        for q in range(NQ):
            qrows = chunk[q * BPQ : (q + 1) * BPQ]  # [BPQ, K1]
            uniq = np.unique(qrows)
            remap = np.searchsorted(uniq, qrows)  # [BPQ, K1]
            ypos[q * BPQ : (q + 1) * BPQ, 0] = np.searchsorted(
                uniq, ychunk[q * BPQ : (q + 1) * BPQ]
            )
            t = np.zeros((TQ, D2), np.float32)
            t[: len(uniq), :D] = memory_l[uniq]
            t[: len(uniq), D:] = memory_ab[uniq]
            m[f"table_{q}"] = t
            for j in range(BPQ):
                b = q * BPQ + j
                padded = np.zeros(KPAD, np.int64)
                padded[:K1] = remap[j]
                idx16[:, b * SWRAP : (b + 1) * SWRAP] = wrap_idx(padded)
        # feature pairs in gather-row order: [ab | l] per query
        fcat = np.concatenate([ab[bs], l[bs]], axis=1)  # [BPC, 2D] = [ab | l]
        frep = np.broadcast_to(fcat.reshape(1, BPC * D2), (P, BPC * D2))
        m.update(
            {
                "idx16": idx16,
                "frep": np.ascontiguousarray(frep),
                "l16": np.ascontiguousarray(l[bs]),
                "ab16": np.ascontiguousarray(ab[bs]),
                "yidx": ypos,
            }
        )
        in_maps.append(m)
    return in_maps


def assemble(results, y, memory_l, memory_ab):
    out_l = np.empty((B, K1, 1), np.float32)
    out_ab = np.empty((B, K1, 1), np.float32)
    new_memory_l = memory_l.copy()
    new_memory_ab = memory_ab.copy()
    for c in range(NCORES):
        bs = slice(c * BPC, (c + 1) * BPC)
        for key, dst in (("score_l", out_l), ("score_ab", out_ab)):
            r = results[c][key]  # [P, KCOLS]
            t = r.reshape(P, BPC, KT).transpose(1, 2, 0).reshape(BPC, KPAD)[:, :K1]
            dst[bs] = t[:, :, None]
        new_memory_l[y[bs]] = results[c]["upd_l"]
        new_memory_ab[y[bs]] = results[c]["upd_ab"]
    return out_l, out_ab, new_memory_l, new_memory_ab


def kernel(l, ab, y, idx, memory_l, memory_ab, trace=False, _return_raw=False):
    l = np.ascontiguousarray(np.asarray(l), dtype=np.float32)
    ab = np.ascontiguousarray(np.asarray(ab), dtype=np.float32)
    y = np.ascontiguousarray(np.asarray(y), dtype=np.int32)
    idx = np.ascontiguousarray(np.asarray(idx), dtype=np.int32)
    memory_l = np.ascontiguousarray(np.asarray(memory_l), dtype=np.float32)
    memory_ab = np.ascontiguousarray(np.asarray(memory_ab), dtype=np.float32)

    nc = _get_nc()
    in_maps = make_in_maps(l, ab, y, idx, memory_l, memory_ab)
    br = run_bass_kernel_spmd(nc, in_maps, core_ids=list(range(NCORES)), trace=trace)
    out = assemble(br.results, y, memory_l, memory_ab)
    if _return_raw:
        return out, br
    return out
